# revision 6
# baseline (speedup 1.0000x reference)
"""TRN2 Bass kernel for multi-head self-attention with RoPE (causal).

Problem: B=4, S=2048, D=768, H=12 heads of dk=64, fp32 in/out.

Sharding: 8 cores = 4 batches x 2 head-groups of 6 heads. Each core computes
QKV projections for its 6 heads, RoPE, causal attention, and a partial
output projection; the host sums the two partials per batch.

Numerics: split-bf16 (hi+lo) 3-term matmuls for the Q/K projections and
for Q.K^T scores (the softmax here is argmax-like: scaled-score std ~96,
top-gap ~25, so bf16/tf32 rounding would flip winners). V/AV/O in bf16.

v2 changes vs baseline:
  - per-row max via chained nc.vector.tensor_mask_reduce (fuses causal
    mask + 1/8 scale + running max + negation -> exp bias in one DVE op
    per chunk); both chunks stay resident in PSUM so one exp pass with
    the final bias replaces the flash merge machinery entirely.
  - softmax denominator via exp accum_out; per-head normalization uses a
    PE transpose of the denominators + K=1 broadcast matmuls instead of
    gpsimd partition_broadcast + per-tile DMA gathers.
  - RoPE cos/sin multiplies run on the (otherwise idle) GpSimd engine.
"""

import sys

sys.path.insert(0, "/opt/trn_rl_repo")

from contextlib import ExitStack

import ml_dtypes
import numpy as np

import concourse.bass as bass
import concourse.tile as tile
from concourse import bacc, mybir
from concourse.bass_utils import run_bass_kernel_spmd

F32 = mybir.dt.float32
BF16 = mybir.dt.bfloat16
bf16 = ml_dtypes.bfloat16

B, D, H, DK = 4, 768, 12, 64
NHC = 6          # heads per core
NPAIR = 3        # head pairs per core
DSUB = 6         # d_in subtiles of 128
CPC = NHC * DK   # 384 head-dims per core


def _build(S=2048, CHUNK=1024, trace_label="", debug_stop=""):
    """Build the SPMD program (pair-pipelined: proj of pair p+1 overlaps
    attention of pair p, keeping the PE dense so HAM stays at full clock)."""
    NQT = S // 128
    nc = bacc.Bacc("TRN2", target_bir_lowering=False, debug=False, num_devices=8)

    def din(name, shape, dt):
        return nc.dram_tensor(name, shape, dt, kind="ExternalInput").ap()

    xh_d = din("xh", [128, DSUB, S], BF16)
    xl_d = din("xl", [128, DSUB, S], BF16)
    wqh_d = din("wqh", [128, DSUB, CPC], BF16)
    wql_d = din("wql", [128, DSUB, CPC], BF16)
    wkh_d = din("wkh", [128, DSUB, CPC], BF16)
    wkl_d = din("wkl", [128, DSUB, CPC], BF16)
    wv_d = din("wvT", [128, DSUB, CPC], BF16)
    wo_d = din("woT", [128, NPAIR, D], BF16)
    cos_d = din("cos_t", [128, S], F32)
    sin_d = din("sin_t", [128, S], F32)
    mask_d = din("mask", [128, 128], F32)
    id_d = din("ident", [128, 128], F32)
    out_d = nc.dram_tensor("out", [S, D], F32, kind="ExternalOutput").ap()

    MAXOP = mybir.AluOpType.max

    with tile.TileContext(nc) as tc, ExitStack() as ctx:
        # ---------- persistent SBUF ----------
        pers = ctx.enter_context(tc.tile_pool(name="pers", bufs=1))

        def load(pool, dr, name):
            t = pool.tile(list(dr.shape), dr.dtype, tag=f"L{name}")
            nc.sync.dma_start(t[:], dr[:])
            return t

        wo = load(pers, wo_d, "wo")
        mask = load(pers, mask_d, "mask")
        ident = load(pers, id_d, "id")

        # band layouts (all matmul operands at base partition 0):
        # q_hl: band0 = q_hi, band1 = q_lo; k_hh: k_hi in both bands;
        # k_l: k_lo on partitions 0:64
        q_hl = pers.tile([128, NHC, S], BF16, tag="q_hl")
        k_hh = pers.tile([128, NHC, S], BF16, tag="k_hh")
        k_l = pers.tile([64, NHC, S], BF16, tag="k_l")
        v_sb = pers.tile([128, NQT, CPC], BF16, tag="v_sb")
        # unnormalized avT in O-proj lhsT layout (normalized in place later)
        av_all = pers.tile([128, NPAIR, S], BF16, tag="av_all")
        den_a = pers.tile([128, NHC, NQT], F32, tag="den_a")
        den_b = pers.tile([128, NHC, NQT], F32, tag="den_b")

        with tc.tile_pool(name="bload", bufs=1) as bl, \
             tc.tile_pool(name="bx", bufs=2) as bx, \
             tc.tile_pool(name="projwork", bufs=2) as pwk, \
             tc.tile_pool(name="projpsum", bufs=1, space="PSUM") as pps, \
             tc.tile_pool(name="scps", bufs=2, space="PSUM") as scps, \
             tc.tile_pool(name="avps", bufs=2, space="PSUM") as avps, \
             tc.tile_pool(name="atwork", bufs=2) as awk, \
             tc.tile_pool(name="norm", bufs=1) as nwk, \
             tc.tile_pool(name="stats", bufs=8) as stp:

            nc.vector.memset(den_b[:].rearrange("p a b -> p (a b)"), 0.0)

            wqh = load(bl, wqh_d, "wqh")
            wql = load(bl, wql_d, "wql")
            wkh = load(bl, wkh_d, "wkh")
            wkl = load(bl, wkl_d, "wkl")
            wv = load(bl, wv_d, "wv")
            cos_t = load(bl, cos_d, "cos")
            sin_t = load(bl, sin_d, "sin")

            # ---- V projection first (dense matmuls warm the PE) ----
            for sc_i in range(S // 512):
                ssl = bass.ts(sc_i, 512)
                xv = bx.tile([128, 2, DSUB, 512], BF16, tag="xc")
                nc.sync.dma_start(xv[:, 0], xh_d[:, :, ssl])
                for st4 in range(4):
                    st = sc_i * 4 + st4
                    psv = pps.tile([128, 2, 512], F32, tag="pp")
                    for t in range(DSUB):
                        nc.tensor.matmul(
                            psv[:, 0, 0:CPC],
                            xv[:, 0, t, bass.ts(st4, 128)], wv[:, t, :],
                            start=(t == 0), stop=(t == DSUB - 1),
                        )
                    nc.scalar.copy(out=v_sb[:, st, :], in_=psv[:, 0, 0:CPC])

            def proj_pair(p):
                for sc_i in range(S // 512):
                    ssl = bass.ts(sc_i, 512)
                    xc = bx.tile([128, 2, DSUB, 512], BF16, tag="xc")
                    nc.sync.dma_start(xc[:, 0], xh_d[:, :, ssl])
                    nc.sync.dma_start(xc[:, 1], xl_d[:, :, ssl])
                    pqk = pps.tile([128, 2, 512], F32, tag="pp")
                    for qk, (w_hi, w_lo) in enumerate(
                        ((wqh, wql), (wkh, wkl))
                    ):
                        n = 0
                        for t in range(DSUB):
                            for lh, xi in ((w_hi, 0), (w_hi, 1), (w_lo, 0)):
                                nc.tensor.matmul(
                                    pqk[:, qk, :],
                                    lh[:, t, bass.ts(p, 128)],
                                    xc[:, xi, t, :],
                                    start=(n == 0), stop=(n == 3 * DSUB - 1),
                                )
                                n += 1
                    # rope (2 heads stacked on partitions)
                    for qk in range(2):
                        f32c = pwk.tile([128, 512], F32, tag="f32c")
                        nc.scalar.copy(out=f32c[:], in_=pqk[:, qk, :])
                        swp = pwk.tile([128, 512], F32, tag="swp")
                        for a in range(2):
                            nc.sync.dma_start(
                                swp[64 * a:64 * a + 32, :],
                                f32c[64 * a + 32:64 * a + 64, :],
                            )
                            nc.sync.dma_start(
                                swp[64 * a + 32:64 * a + 64, :],
                                f32c[64 * a:64 * a + 32, :],
                            )
                        m1 = pwk.tile([128, 512], F32, tag="m1")
                        nc.gpsimd.tensor_mul(m1[:], f32c[:], cos_t[:, ssl])
                        rot = pwk.tile([128, 512], F32, tag="rot")
                        nc.gpsimd.tensor_mul(rot[:], swp[:], sin_t[:, ssl])
                        nc.gpsimd.tensor_add(rot[:], rot[:], m1[:])
                        for sub in range(2):
                            hh = 2 * p + sub
                            band = rot[64 * sub:64 * sub + 64, :]
                            if sub == 0:
                                b0 = band
                            else:
                                b0t = pwk.tile([64, 512], F32, tag="m1")
                                nc.vector.tensor_copy(b0t[:], band)
                                b0 = b0t[:]
                            if qk == 0:
                                nc.scalar.copy(
                                    out=q_hl[0:64, hh, ssl], in_=b0)
                                nc.vector.tensor_tensor(
                                    q_hl[64:128, hh, ssl], b0,
                                    q_hl[0:64, hh, ssl],
                                    mybir.AluOpType.subtract,
                                )
                            else:
                                nc.scalar.copy(
                                    out=k_hh[0:64, hh, ssl], in_=b0)
                                nc.vector.tensor_copy(
                                    k_hh[64:128, hh, ssl], b0)
                                nc.vector.tensor_tensor(
                                    k_l[0:64, hh, ssl], b0,
                                    k_hh[0:64, hh, ssl],
                                    mybir.AluOpType.subtract,
                                )

            def attn_head(hh, qt):
                nk = (qt + 1) * 128
                qsl = bass.ts(qt, 128)
                chunks = []
                k0 = 0
                while k0 < nk:
                    chunks.append((k0, min(CHUNK, nk - k0)))
                    k0 += CHUNK
                nch = len(chunks)
                # scores matmuls + fused mask/scale/max (chained) per chunk
                scs = []
                mx_prev = None
                for ci, (k0, nkc) in enumerate(chunks):
                    sc = scps.tile([128, CHUNK], F32, tag="sc")
                    for n0 in range(0, nkc, 512):
                        nn = min(512, nkc - n0)
                        ksl = bass.ds(k0 + n0, nn)
                        nc.tensor.matmul(
                            sc[:, bass.ds(n0, nn)],
                            q_hl[:, hh, qsl], k_hh[:, hh, ksl],
                            start=True, stop=False,
                        )
                        nc.tensor.matmul(
                            sc[:, bass.ds(n0, nn)],
                            q_hl[0:64, hh, qsl], k_l[:, hh, ksl],
                            start=False, stop=True,
                        )
                    last = ci == nch - 1
                    if last:
                        doff = nk - 128 - k0
                        nc.vector.tensor_add(
                            sc[:, bass.ds(doff, 128)],
                            sc[:, bass.ds(doff, 128)], mask[:],
                        )
                    mx = stp.tile([128, 1], F32, tag=f"mx{ci}")
                    nc.vector.tensor_reduce(
                        mx[:], sc[:, 0:nkc], mybir.AxisListType.X, MAXOP,
                    )
                    if ci > 0:
                        mxf = stp.tile([128, 1], F32, tag=f"mxf{ci}")
                        nc.vector.tensor_tensor(
                            mxf[:], mx[:], mx_prev[:], MAXOP)
                        mx = mxf
                    mx_prev = mx
                    scs.append((sc, k0, nkc))
                nbias = stp.tile([128, 1], F32, tag="nbias")
                nc.vector.tensor_scalar_mul(nbias[:], mx_prev[:], -0.125)
                # one exp pass with the final bias + AV accumulation
                avt = avps.tile([128, 512], F32, tag="avt")
                for ci, (sc, k0, nkc) in enumerate(scs):
                    dden = (den_a if ci == 0 else den_b)[:, hh, qt:qt + 1]
                    psb = awk.tile([128, CHUNK], BF16, tag="psb")
                    nc.scalar.activation(
                        psb[:, 0:nkc], sc[:, 0:nkc],
                        mybir.ActivationFunctionType.Exp,
                        bias=nbias[:, 0:1], scale=0.125,
                        accum_out=dden,
                    )
                    pts = awk.tile([128, CHUNK // 128, 128], BF16, tag="pts")
                    nblk = nkc // 128
                    nc.sync.dma_start_transpose(pts[:, 0:nblk, :],
                                                psb[:, 0:nkc])
                    for j in range(nblk):
                        kt = (k0 + j * 128) // 128
                        nc.tensor.matmul(
                            avt[0:64, 0:128],
                            v_sb[:, kt, bass.ds(64 * hh, 64)],
                            pts[:, j, :],
                            start=(ci == 0 and j == 0),
                            stop=(ci == nch - 1 and j == nblk - 1),
                            skip_group_check=True,
                        )
                p_, h_ = hh // 2, hh % 2
                nc.scalar.copy(
                    out=av_all[64 * h_:64 * h_ + 64, p_, qsl],
                    in_=avt[0:64, 0:128])

            def normalize_head(hh):
                # den = den_a + den_b; rec = 1/den; PE-transpose rec so one
                # 16-descriptor DMA gathers it to a [1, S] row; gpsimd
                # broadcasts along partitions; one DVE mul scales av.
                p_, h_ = hh // 2, hh % 2
                denf = stp.tile([128, NQT], F32, tag="denf")
                nc.vector.tensor_add(
                    denf[:], den_a[:, hh, :], den_b[:, hh, :])
                rec = stp.tile([128, NQT], F32, tag="rec")
                nc.vector.reciprocal(rec[:], denf[:])
                rT = avps.tile([128, 512], F32, tag="avt")
                nc.tensor.transpose(rT[0:NQT, 0:128], rec[:], ident[:])
                rTs = stp.tile([NQT, 128], F32, tag="rTs")
                nc.scalar.copy(out=rTs[:], in_=rT[0:NQT, 0:128])
                for half in range(2):
                    hq = NQT // 2
                    rech = nwk.tile([1, S // 2], F32, tag="rech")
                    nc.sync.dma_start(
                        rech[0:1, :], rTs[half * hq:(half + 1) * hq, :])
                    recb = nwk.tile([128, S // 2], F32, tag="recb")
                    nc.gpsimd.partition_broadcast(recb[:], rech[0:1, :])
                    sl = av_all[64 * h_:64 * h_ + 64, p_,
                                bass.ts(half, S // 2)]
                    nc.vector.tensor_mul(
                        sl, sl, recb[64 * h_:64 * h_ + 64, :])

            # ---- pair-pipelined main loop ----
            for step in range(NPAIR + 1):
                if step < NPAIR:
                    proj_pair(step)
                if step > 0:
                    for sub in range(2):
                        hh = 2 * (step - 1) + sub
                        for qt in range(NQT):
                            attn_head(hh, qt)
                        normalize_head(hh)

        # ---------- output projection ----------
        with tc.tile_pool(name="ops", bufs=2, space="PSUM") as ops, \
             tc.tile_pool(name="owork", bufs=3) as owk:
            for st in range(NQT):
                po = ops.tile([128, 2, 512], F32, tag="po")
                for half in range(2):
                    for p in range(NPAIR):
                        nc.tensor.matmul(
                            po[:, half, 0:384],
                            av_all[:, p, bass.ts(st, 128)],
                            wo[:, p, bass.ts(half, 384)],
                            start=(p == 0), stop=(p == NPAIR - 1),
                        )
                osb = owk.tile([128, D], F32, tag="osb")
                nc.scalar.copy(out=osb[:, 0:384], in_=po[:, 0, 0:384])
                nc.scalar.copy(out=osb[:, 384:768], in_=po[:, 1, 0:384])
                nc.sync.dma_start(out_d[bass.ts(st, 128), :], osb[:])

    nc.compile()
    return nc


def _rope_perm():
    p = np.zeros(DK, dtype=np.int64)
    for i in range(DK // 2):
        p[i] = 2 * i
        p[i + 32] = 2 * i + 1
    return p


def _split(a):
    hi = a.astype(bf16)
    lo = (a.astype(np.float32) - hi.astype(np.float32)).astype(bf16)
    return hi, lo


def _tile_din(a):
    # [768, F] -> [128, 6, F]
    return np.ascontiguousarray(a.reshape(DSUB, 128, -1).transpose(1, 0, 2))


def make_inputs(x, wq, wk, wv, wo, S):
    """Host-side prep: returns list of 8 in_maps (core = 2*b + g)."""
    NQT = S // 128
    perm = _rope_perm()
    pos = np.arange(S, dtype=np.float64)
    inv = 10000.0 ** (-2.0 * np.arange(DK // 2, dtype=np.float64) / DK)
    ang = pos[:, None] * inv[None, :]
    cosv = np.cos(ang).astype(np.float32).T  # [32, S]
    sinv = np.sin(ang).astype(np.float32).T
    cos_t = np.tile(cosv, (4, 1)).astype(np.float32)            # [128, S]
    sin_t = np.tile(
        np.concatenate([-sinv, sinv], axis=0), (2, 1)
    ).astype(np.float32)                                        # [128, S]

    mask = np.triu(np.full((128, 128), -1e9, np.float32), 1)
    ident = np.eye(128, dtype=np.float32)

    maps = []
    for b in range(B):
        xT = np.ascontiguousarray(x[b].T.astype(np.float32))  # [768, S]
        xh, xl = _split(xT)
        xh_t, xl_t = _tile_din(xh), _tile_din(xl)
        for g in range(2):
            hs = slice(g * CPC, (g + 1) * CPC)
            wqc = wq[hs].astype(np.float32).copy()
            wkc = wk[hs].astype(np.float32).copy()
            for arr in (wqc, wkc):
                for i in range(NHC):
                    blk = arr[i * DK:(i + 1) * DK].copy()
                    arr[i * DK:(i + 1) * DK] = blk[perm]
            wqh, wql = _split(wqc.T)  # [768, 384]
            wkh, wkl = _split(wkc.T)
            wvT = wv[hs].astype(np.float32).T.astype(bf16)
            woT = wo[:, hs].astype(np.float32).T.astype(bf16)  # [384, 768]
            maps.append({
                "xh": xh_t, "xl": xl_t,
                "wqh": _tile_din(wqh), "wql": _tile_din(wql),
                "wkh": _tile_din(wkh), "wkl": _tile_din(wkl),
                "wvT": _tile_din(wvT),
                "woT": np.ascontiguousarray(
                    woT.reshape(NPAIR, 128, D).transpose(1, 0, 2)),
                "cos_t": cos_t, "sin_t": sin_t,
                "mask": mask, "ident": ident,
            })
    return maps


_PROG = {}


def _prog(S, CHUNK):
    key = (S, CHUNK)
    if key not in _PROG:
        _PROG[key] = _build(S, CHUNK)
    return _PROG[key]


def kernel(x, wq, wk, wv, wo, S=2048, CHUNK=1024, trace=False):
    x = np.asarray(x, np.float32)
    nc = _prog(S, CHUNK)
    maps = make_inputs(x, np.asarray(wq), np.asarray(wk), np.asarray(wv),
                       np.asarray(wo), S)
    res = run_bass_kernel_spmd(nc, maps, list(range(8)), trace=trace)
    outs = []
    for b in range(B):
        outs.append(res.results[2 * b]["out"] + res.results[2 * b + 1]["out"])
    out = np.stack(outs)
    if trace:
        kernel.last_exec_time_ns = res.exec_time_ns
        kernel.last_results = res
    return out
